# revision 20
# baseline (speedup 1.0000x reference)
"""Trainium2 Bass kernel for nn_MiniAttentionBlock.

Reference computation (B=16, S=4096, F=512):
    h      = tanh(x @ W + b)        [B,S,F]
    scores = h @ u                  [B,S]
    e      = exp(scores)
    a      = e / (sum(e) + eps)     global normalization over all B*S
    out    = sum_s x[b,s,:] * a[b,s]  -> [B,F]

Strategy: data-parallel over batch, 2 batches per core (8 cores).  The
denominator sum(e) is a single small AllReduce.  Each core receives its
x shard pre-transposed to [F, R] (R = 2*4096 rows) as a single bf16
copy feeding both the TensorE matmul path and the VectorE pooling path
(the 2e-2 rel-err budget comfortably absorbs bf16 pooling noise).

Per 512-row block:
  - h^T[g, rows] accumulated over 4 k-chunks of bf16 matmuls, two
    m-chunks per 2-bank PSUM tile (3 tiles rotating)
  - tanh fused over each m-chunk pair on ScalarE, PSUM -> SBUF (bf16):
    halves the ScalarE per-op overhead vs per-chunk tanh, keeping
    ScalarE well under the TensorE per-block budget so the PE never
    waits on activations (bias is applied per-chunk only when b != 0;
    the problem spec fixes b = 0)
  - the scores matmul runs TWO BLOCKS behind the h matmuls, so its
    dependency on the tanh output can never head-of-line-block the
    TensorE queue; u replicated 128x along the stationary free dim makes
    the PSUM result [128, rows] carry scores broadcast to all partitions
  - exp on ScalarE gives the e-broadcast tile directly, with the
    block's partial sum accumulated for free (accum_out)
  - weighted pooling sum_rows e*x runs on VectorE: fused
    multiply+row-reduce custom DVE op, x in bf16
The AllReduce carries the scalar denominator replicated on 128
partitions so the reciprocal applies per-partition with no
partition-broadcast afterwards; only the last two blocks' pooling is
deferred past the collective issue so the DVE FIFO never delays the
denominator AllGather.
"""

import sys

if "/opt/trn_rl_repo" not in sys.path:
    sys.path.insert(0, "/opt/trn_rl_repo")

import numpy as np
import ml_dtypes

from concourse import bass, bacc, tile, bass_utils
from concourse.dve_ops import TENSOR_TENSOR_REDUCE

mybir = bass.mybir

B, S, F = 16, 4096, 512
N_CORES = 8
BPC = B // N_CORES          # batches per core
R = BPC * S                 # rows per core
RB = 512                    # rows per block
NBLK = R // RB              # blocks per core
DEFER = 1                   # trailing blocks whose pooling overlaps the AllReduce
LAG = 2                     # blocks the scores matmul trails the h matmul
NKC = F // 128              # 128-partition chunks of F
CB = NKC * RB               # columns per block of h (2048)
HB = 2 * RB                 # columns per h PSUM tile (2 m-chunks)
EPS = 1e-7

F32 = mybir.dt.float32
BF16 = mybir.dt.bfloat16
ALU = mybir.AluOpType
ACTF = mybir.ActivationFunctionType
AXIS = mybir.AxisListType

_CACHE = {}


def _build(use_bias: bool):
    nc = bacc.Bacc("TRN2", target_bir_lowering=False, debug=False,
                   num_devices=N_CORES)

    xb = nc.dram_tensor("xb", [NBLK * 128, CB], BF16, kind="ExternalInput")
    w = nc.dram_tensor("w", [F, F], BF16, kind="ExternalInput")
    b2 = nc.dram_tensor("b2", [128, NKC], F32, kind="ExternalInput")
    ur = nc.dram_tensor("ur", [128, NKC, 128], BF16, kind="ExternalInput")
    out = nc.dram_tensor("out", [128, BPC * NKC], F32, kind="ExternalOutput")

    with tile.TileContext(nc) as tc:
        with tc.tile_pool(name="const", bufs=1) as cpool, \
             tc.tile_pool(name="xbp", bufs=1) as xbp, \
             tc.tile_pool(name="hap", bufs=LAG + 2) as hap, \
             tc.tile_pool(name="erp", bufs=DEFER + 2) as erp, \
             tc.tile_pool(name="scr", bufs=3) as scr, \
             tc.tile_pool(name="hps", bufs=3, space="PSUM") as hps, \
             tc.tile_pool(name="sps", bufs=2, space="PSUM") as sps, \
             tc.tile_pool(name="dram", bufs=1, space="DRAM") as dram:

            # ---- constants ----
            w_sb = []
            for kc in range(NKC):
                t = cpool.tile([128, F], BF16, tag=f"w{kc}")
                nc.scalar.dma_start(out=t[:], in_=w.ap()[kc * 128:(kc + 1) * 128, :])
                w_sb.append(t)
            b_sb = cpool.tile([128, NKC], F32, tag="b")
            nc.scalar.dma_start(out=b_sb[:], in_=b2.ap())
            u_sb = cpool.tile([128, NKC, 128], BF16, tag="u")
            nc.scalar.dma_start(out=u_sb[:], in_=ur.ap())

            # warmup collective: pre-warms the ncfw/credit machinery while
            # compute runs; its result is unused
            wu_in = dram.tile([1, 8], F32)
            wu_out = dram.tile([8, 8], F32, addr_space="Shared")
            wu_sb = cpool.tile([1, 8], F32, tag="wusb")
            nc.vector.memset(wu_sb[:], 0.0)
            nc.scalar.dma_start(out=wu_in[:], in_=wu_sb[:])
            nc.gpsimd.collective_compute(
                "AllGather", ALU.bypass,
                replica_groups=[list(range(N_CORES))],
                ins=[wu_in.opt()], outs=[wu_out.opt()])
            ones8 = cpool.tile([8, 128], F32, tag="ones8")
            nc.vector.memset(ones8[:], 1.0)

            esum = cpool.tile([128, NBLK], F32, tag="esum")
            nums = [cpool.tile([128, NBLK], F32, tag=f"num{kc}", name=f"num{kc}")
                    for kc in range(NKC)]
            out_sb = cpool.tile([128, BPC, NKC], F32, tag="osb")

            # [128, NBLK, CB]: partition-major view of the per-block stacking
            xb_v = xb.ap().rearrange("(n p) c -> p n c", p=128)

            # ---- prefetch the full x shard; DMA engines go quiet well
            # before the denominator collective ----
            hacts = {}   # blk -> hact tile
            xbss = {}    # blk -> chunk APs
            ers = {}     # blk -> e tile
            for blk in range(NBLK):
                xball = xbp.tile([128, NKC, RB], BF16, tag=f"xb{blk}",
                                 name=f"xb{blk}")
                if blk == 0:
                    # per-chunk loads so the first matmul starts on chunk 0
                    for kc in range(NKC):
                        nc.sync.dma_start(
                            out=xball[:, kc, :],
                            in_=xb_v[:, 0, kc * RB:(kc + 1) * RB])
                else:
                    nc.sync.dma_start(
                        out=xball[:],
                        in_=xb_v[:, blk, :])
                xbss[blk] = [xball[:, kc, :] for kc in range(NKC)]

            def emit_scores(blk):
                # scores broadcast to all 128 partitions via replicated u
                sp = sps.tile([128, RB], F32, tag="s", name="sp")
                hact = hacts.pop(blk)
                for mc in range(NKC):
                    nc.tensor.matmul(
                        sp[:],
                        lhsT=u_sb[:, mc, :],
                        rhs=hact[:, mc * RB:(mc + 1) * RB],
                        start=(mc == 0), stop=(mc == NKC - 1))
                # e (broadcast) = exp(scores); block partial sum for free
                er = erp.tile([128, RB], F32, tag="er", name="er")
                nc.scalar.activation(out=er[:], in_=sp[:], func=ACTF.Exp,
                                     accum_out=esum[:, blk:blk + 1])
                ers[blk] = er

            def emit_pool(blk):
                # num[f] += sum_rows xT[f, row] * e[row]
                er = ers.pop(blk)
                xbs = xbss.pop(blk)
                for kc in range(NKC):
                    sc = scr.tile([128, RB], F32, tag="sc", name="sc")
                    nc.vector._custom_dve(
                        TENSOR_TENSOR_REDUCE,
                        out=sc[:], in0=xbs[kc], in1=er[:],
                        s0=0.0, s1=1.0,
                        accum_out=nums[kc][:, blk:blk + 1])

            for blk in range(NBLK):
                # h^T[g, rows] = sum_f W[f, g] * xT[f, rows]; tanh per
                # m-chunk pair
                hact = hap.tile([128, CB], BF16, tag="h")
                for pair in range(NKC // 2):
                    ps = hps.tile([128, HB], F32, tag="h")
                    for sub in range(2):
                        mc = pair * 2 + sub
                        for kc in range(NKC):
                            nc.tensor.matmul(
                                ps[:, sub * RB:(sub + 1) * RB],
                                lhsT=w_sb[kc][:, mc * 128:(mc + 1) * 128],
                                rhs=xbss[blk][kc],
                                start=(kc == 0), stop=(kc == NKC - 1))
                    if use_bias:
                        for sub in range(2):
                            mc = pair * 2 + sub
                            nc.scalar.activation(
                                out=hact[:, mc * RB:(mc + 1) * RB],
                                in_=ps[:, sub * RB:(sub + 1) * RB],
                                func=ACTF.Tanh, bias=b_sb[:, mc:mc + 1])
                    else:
                        # b is all-zero per the problem spec
                        nc.scalar.activation(
                            out=hact[:, pair * HB:(pair + 1) * HB],
                            in_=ps[:], func=ACTF.Tanh, bias=b_sb[:, 0:1])
                hacts[blk] = hact

                # scores/exp/pooling trail LAG blocks behind the h matmuls
                if blk >= LAG:
                    emit_scores(blk - LAG)
                    if blk - LAG < NBLK - DEFER:
                        emit_pool(blk - LAG)

            for blk in range(NBLK - LAG, NBLK):
                emit_scores(blk)
                if blk < NBLK - DEFER:
                    emit_pool(blk)

            # ---- finalize ----
            # local sum, replicated on all 128 partitions
            s_loc = cpool.tile([128, 1], F32, tag="sloc")
            nc.vector.tensor_reduce(out=s_loc[:], in_=esum[:],
                                    axis=AXIS.X, op=ALU.add)

            cc_in = dram.tile([1, 8], F32)
            cc_out = dram.tile([8, 8], F32, addr_space="Shared")
            s8 = cpool.tile([1, 8], F32, tag="s8")
            nc.vector.tensor_scalar_add(out=s8[:], in0=wu_sb[:],
                                        scalar1=s_loc[0:1, 0:1])
            nc.sync.dma_start(out=cc_in[:], in_=s8[:])
            nc.gpsimd.collective_compute(
                "AllGather", ALU.bypass,
                replica_groups=[list(range(N_CORES))],
                ins=[cc_in.opt()], outs=[cc_out.opt()])
            sg8 = cpool.tile([8, 8], F32, tag="sg8")
            nc.sync.dma_start(out=sg8[:], in_=cc_out[:])

            # pooling for the deferred blocks, overlapping the AllReduce
            for blk in range(NBLK - DEFER, NBLK):
                emit_pool(blk)

            # unnormalized per-batch sums (independent of the collective)
            for bb in range(BPC):
                for kc in range(NKC):
                    nc.vector.tensor_reduce(
                        out=out_sb[:, bb, kc:kc + 1],
                        in_=nums[kc][:, bb * (NBLK // BPC):(bb + 1) * (NBLK // BPC)],
                        axis=AXIS.X, op=ALU.add)

            # combine the 8 gathered partials and broadcast to 128 partitions
            # in one small fp32 matmul: ones8^T[128,8] @ sg8[:,0] -> [128,1]
            psg = sps.tile([128, RB], F32, tag="s")
            nc.tensor.matmul(psg[:, 0:1], lhsT=ones8[:], rhs=sg8[:, 0:1],
                             start=True, stop=True)
            sg = cpool.tile([128, 1], F32, tag="sg")
            nc.scalar.copy(out=sg[:], in_=psg[:, 0:1])

            rcp = cpool.tile([128, 1], F32, tag="rcp")
            nc.vector.tensor_scalar_add(out=rcp[:], in0=sg[:], scalar1=EPS)
            nc.vector.reciprocal(out=rcp[:], in_=rcp[:])
            nc.vector.tensor_scalar_mul(out=out_sb[:], in0=out_sb[:],
                                        scalar1=rcp[:])

            nc.sync.dma_start(out=out.ap(), in_=out_sb[:])

    nc.compile()
    return nc


def _get_compiled(use_bias: bool):
    key = ("nc", use_bias)
    if key not in _CACHE:
        _CACHE[key] = _build(use_bias)
    return _CACHE[key]


def _make_in_maps(x, W, b, u):
    Wc = np.ascontiguousarray(
        np.asarray(W, np.float32).astype(ml_dtypes.bfloat16))
    bc = np.ascontiguousarray(np.asarray(b, np.float32).reshape(NKC, 128).T)
    u_cols = np.asarray(u, np.float32).reshape(NKC, 128).T  # [128, NKC]
    urc = np.ascontiguousarray(
        np.broadcast_to(u_cols[:, :, None], (128, NKC, 128))
    ).astype(ml_dtypes.bfloat16)
    in_maps = []
    for c in range(N_CORES):
        xs = np.asarray(x[BPC * c:BPC * (c + 1)], np.float32).reshape(R, F).T
        xsb = np.ascontiguousarray(
            xs.reshape(NKC, 128, NBLK, RB).transpose(2, 1, 0, 3)
        ).astype(ml_dtypes.bfloat16).reshape(NBLK * 128, CB)
        in_maps.append({"xb": xsb, "w": Wc, "b2": bc, "ur": urc})
    return in_maps


def _unshard_out(o):
    # [128, BPC, NKC] -> [BPC, F]: out[b, kc*128+p] = o[p, b, kc]
    return np.ascontiguousarray(
        o.reshape(128, BPC, NKC).transpose(1, 2, 0).reshape(BPC, F))


def kernel(x, W, b, u):
    use_bias = bool(np.any(np.asarray(b, np.float32)))
    nc = _get_compiled(use_bias)
    in_maps = _make_in_maps(x, W, b, u)
    res = bass_utils.run_bass_kernel_spmd(
        nc, in_maps, core_ids=list(range(N_CORES)))
    _CACHE["last_results"] = res
    return np.concatenate([_unshard_out(res.results[c]["out"])
                           for c in range(N_CORES)], axis=0)


def kernel_traced(x, W, b, u, **trace_kwargs):
    """Same as kernel() but with NTFF tracing; returns (out, BassKernelResults)."""
    use_bias = bool(np.any(np.asarray(b, np.float32)))
    nc = _get_compiled(use_bias)
    in_maps = _make_in_maps(x, W, b, u)
    res = bass_utils.run_bass_kernel_spmd(
        nc, in_maps, core_ids=list(range(N_CORES)), trace=True, **trace_kwargs)
    _CACHE["last_results"] = res
    out = np.concatenate([_unshard_out(res.results[c]["out"])
                          for c in range(N_CORES)], axis=0)
    return out, res


# revision 21
# speedup vs baseline: 1.5201x; 1.5201x over previous
"""Trainium2 Bass kernel for nn_MiniAttentionBlock.

Reference computation (B=16, S=4096, F=512):
    h      = tanh(x @ W + b)        [B,S,F]
    scores = h @ u                  [B,S]
    e      = exp(scores)
    a      = e / (sum(e) + eps)     global normalization over all B*S
    out    = sum_s x[b,s,:] * a[b,s]  -> [B,F]

Strategy: data-parallel over batch, 2 batches per core (8 cores).  The
denominator sum(e) is a single small AllReduce.  Each core receives its
x shard pre-transposed to [F, R] (R = 2*4096 rows) as a single bf16
copy feeding both the TensorE matmul path and the VectorE pooling path
(the 2e-2 rel-err budget comfortably absorbs bf16 pooling noise).

Per 512-row block:
  - h^T[g, rows] accumulated over 4 k-chunks of bf16 matmuls, two
    m-chunks per 2-bank PSUM tile (3 tiles rotating)
  - tanh fused over each m-chunk pair on ScalarE, PSUM -> SBUF (bf16):
    halves the ScalarE per-op overhead vs per-chunk tanh, keeping
    ScalarE well under the TensorE per-block budget so the PE never
    waits on activations (bias is applied per-chunk only when b != 0;
    the problem spec fixes b = 0)
  - the scores matmul runs TWO BLOCKS behind the h matmuls, so its
    dependency on the tanh output can never head-of-line-block the
    TensorE queue; u replicated 128x along the stationary free dim makes
    the PSUM result [128, rows] carry scores broadcast to all partitions
  - exp on ScalarE gives the e-broadcast tile directly, with the
    block's partial sum accumulated for free (accum_out)
  - weighted pooling sum_rows e*x runs on VectorE: fused
    multiply+row-reduce custom DVE op, x in bf16
The AllReduce carries the scalar denominator replicated on 128
partitions so the reciprocal applies per-partition with no
partition-broadcast afterwards; only the last two blocks' pooling is
deferred past the collective issue so the DVE FIFO never delays the
denominator AllGather.
"""

import sys

if "/opt/trn_rl_repo" not in sys.path:
    sys.path.insert(0, "/opt/trn_rl_repo")

import numpy as np
import ml_dtypes

from concourse import bass, bacc, tile, bass_utils
from concourse.dve_ops import TENSOR_TENSOR_REDUCE

mybir = bass.mybir

B, S, F = 16, 4096, 512
N_CORES = 8
BPC = B // N_CORES          # batches per core
R = BPC * S                 # rows per core
RB = 512                    # rows per block
NBLK = R // RB              # blocks per core
DEFER = 1                   # trailing blocks whose pooling overlaps the AllReduce
LAG = 2                     # blocks the scores matmul trails the h matmul
NKC = F // 128              # 128-partition chunks of F
CB = NKC * RB               # columns per block of h (2048)
HB = 2 * RB                 # columns per h PSUM tile (2 m-chunks)
EPS = 1e-7

F32 = mybir.dt.float32
BF16 = mybir.dt.bfloat16
ALU = mybir.AluOpType
ACTF = mybir.ActivationFunctionType
AXIS = mybir.AxisListType

_CACHE = {}


def _build(use_bias: bool):
    nc = bacc.Bacc("TRN2", target_bir_lowering=False, debug=False,
                   num_devices=N_CORES)

    xb = nc.dram_tensor("xb", [NBLK * 128, CB], BF16, kind="ExternalInput")
    w = nc.dram_tensor("w", [F, F], BF16, kind="ExternalInput")
    b2 = nc.dram_tensor("b2", [128, NKC], F32, kind="ExternalInput")
    ur = nc.dram_tensor("ur", [128, NKC, 128], BF16, kind="ExternalInput")
    out = nc.dram_tensor("out", [128, BPC * NKC], F32, kind="ExternalOutput")

    with tile.TileContext(nc) as tc:
        with tc.tile_pool(name="const", bufs=1) as cpool, \
             tc.tile_pool(name="xbp", bufs=1) as xbp, \
             tc.tile_pool(name="hap", bufs=LAG + 2) as hap, \
             tc.tile_pool(name="erp", bufs=DEFER + 2) as erp, \
             tc.tile_pool(name="scr", bufs=3) as scr, \
             tc.tile_pool(name="hps", bufs=3, space="PSUM") as hps, \
             tc.tile_pool(name="sps", bufs=2, space="PSUM") as sps, \
             tc.tile_pool(name="dram", bufs=1, space="DRAM") as dram:

            # warmup collective FIRST, fed by an uninitialized DRAM tile (its
            # result is unused, so no input prep): every core fires it the
            # moment its gpsimd sequencer is up, so the ncfw/credit init
            # reliably finishes long before the real denominator AllGather
            wu_in = dram.tile([1, 8], F32)
            wu_out = dram.tile([8, 8], F32, addr_space="Shared")
            nc.gpsimd.collective_compute(
                "AllGather", ALU.bypass,
                replica_groups=[list(range(N_CORES))],
                ins=[wu_in.opt()], outs=[wu_out.opt()])

            # ---- constants ----
            w_sb = []
            for kc in range(NKC):
                t = cpool.tile([128, F], BF16, tag=f"w{kc}")
                nc.scalar.dma_start(out=t[:], in_=w.ap()[kc * 128:(kc + 1) * 128, :])
                w_sb.append(t)
            b_sb = cpool.tile([128, NKC], F32, tag="b")
            nc.scalar.dma_start(out=b_sb[:], in_=b2.ap())
            u_sb = cpool.tile([128, NKC, 128], BF16, tag="u")
            nc.scalar.dma_start(out=u_sb[:], in_=ur.ap())

            wu_sb = cpool.tile([1, 8], F32, tag="wusb")
            nc.vector.memset(wu_sb[:], 0.0)
            ones8 = cpool.tile([8, 128], F32, tag="ones8")
            nc.vector.memset(ones8[:], 1.0)

            esum = cpool.tile([128, NBLK], F32, tag="esum")
            nums = [cpool.tile([128, NBLK], F32, tag=f"num{kc}", name=f"num{kc}")
                    for kc in range(NKC)]
            out_sb = cpool.tile([128, BPC, NKC], F32, tag="osb")

            # [128, NBLK, CB]: partition-major view of the per-block stacking
            xb_v = xb.ap().rearrange("(n p) c -> p n c", p=128)

            # ---- prefetch the full x shard; DMA engines go quiet well
            # before the denominator collective ----
            hacts = {}   # blk -> hact tile
            xbss = {}    # blk -> chunk APs
            ers = {}     # blk -> e tile
            for blk in range(NBLK):
                xball = xbp.tile([128, NKC, RB], BF16, tag=f"xb{blk}",
                                 name=f"xb{blk}")
                if blk == 0:
                    # per-chunk loads so the first matmul starts on chunk 0
                    for kc in range(NKC):
                        nc.sync.dma_start(
                            out=xball[:, kc, :],
                            in_=xb_v[:, 0, kc * RB:(kc + 1) * RB])
                else:
                    nc.sync.dma_start(
                        out=xball[:],
                        in_=xb_v[:, blk, :])
                xbss[blk] = [xball[:, kc, :] for kc in range(NKC)]

            def emit_scores(blk):
                # scores broadcast to all 128 partitions via replicated u
                sp = sps.tile([128, RB], F32, tag="s", name="sp")
                hact = hacts.pop(blk)
                for mc in range(NKC):
                    nc.tensor.matmul(
                        sp[:],
                        lhsT=u_sb[:, mc, :],
                        rhs=hact[:, mc * RB:(mc + 1) * RB],
                        start=(mc == 0), stop=(mc == NKC - 1))
                # e (broadcast) = exp(scores); block partial sum for free
                er = erp.tile([128, RB], F32, tag="er", name="er")
                nc.scalar.activation(out=er[:], in_=sp[:], func=ACTF.Exp,
                                     accum_out=esum[:, blk:blk + 1])
                ers[blk] = er

            def emit_pool(blk):
                # num[f] += sum_rows xT[f, row] * e[row]
                er = ers.pop(blk)
                xbs = xbss.pop(blk)
                for kc in range(NKC):
                    sc = scr.tile([128, RB], F32, tag="sc", name="sc")
                    nc.vector._custom_dve(
                        TENSOR_TENSOR_REDUCE,
                        out=sc[:], in0=xbs[kc], in1=er[:],
                        s0=0.0, s1=1.0,
                        accum_out=nums[kc][:, blk:blk + 1])

            for blk in range(NBLK):
                # h^T[g, rows] = sum_f W[f, g] * xT[f, rows]; tanh per
                # m-chunk pair
                hact = hap.tile([128, CB], BF16, tag="h")
                for pair in range(NKC // 2):
                    ps = hps.tile([128, HB], F32, tag="h")
                    for sub in range(2):
                        mc = pair * 2 + sub
                        for kc in range(NKC):
                            nc.tensor.matmul(
                                ps[:, sub * RB:(sub + 1) * RB],
                                lhsT=w_sb[kc][:, mc * 128:(mc + 1) * 128],
                                rhs=xbss[blk][kc],
                                start=(kc == 0), stop=(kc == NKC - 1))
                    if use_bias:
                        for sub in range(2):
                            mc = pair * 2 + sub
                            nc.scalar.activation(
                                out=hact[:, mc * RB:(mc + 1) * RB],
                                in_=ps[:, sub * RB:(sub + 1) * RB],
                                func=ACTF.Tanh, bias=b_sb[:, mc:mc + 1])
                    else:
                        # b is all-zero per the problem spec
                        nc.scalar.activation(
                            out=hact[:, pair * HB:(pair + 1) * HB],
                            in_=ps[:], func=ACTF.Tanh, bias=b_sb[:, 0:1])
                hacts[blk] = hact

                # scores/exp/pooling trail LAG blocks behind the h matmuls
                if blk >= LAG:
                    emit_scores(blk - LAG)
                    if blk - LAG < NBLK - DEFER:
                        emit_pool(blk - LAG)

            for blk in range(NBLK - LAG, NBLK):
                emit_scores(blk)
                if blk < NBLK - DEFER:
                    emit_pool(blk)

            # ---- finalize ----
            # local sum, replicated on all 128 partitions
            s_loc = cpool.tile([128, 1], F32, tag="sloc")
            nc.vector.tensor_reduce(out=s_loc[:], in_=esum[:],
                                    axis=AXIS.X, op=ALU.add)

            cc_in = dram.tile([1, 8], F32)
            cc_out = dram.tile([8, 8], F32, addr_space="Shared")
            s8 = cpool.tile([1, 8], F32, tag="s8")
            nc.vector.tensor_scalar_add(out=s8[:], in0=wu_sb[:],
                                        scalar1=s_loc[0:1, 0:1])
            nc.sync.dma_start(out=cc_in[:], in_=s8[:])
            nc.gpsimd.collective_compute(
                "AllGather", ALU.bypass,
                replica_groups=[list(range(N_CORES))],
                ins=[cc_in.opt()], outs=[cc_out.opt()])
            sg8 = cpool.tile([8, 8], F32, tag="sg8")
            nc.sync.dma_start(out=sg8[:], in_=cc_out[:])

            # pooling for the deferred blocks, overlapping the AllReduce
            for blk in range(NBLK - DEFER, NBLK):
                emit_pool(blk)

            # unnormalized per-batch sums (independent of the collective)
            for bb in range(BPC):
                for kc in range(NKC):
                    nc.vector.tensor_reduce(
                        out=out_sb[:, bb, kc:kc + 1],
                        in_=nums[kc][:, bb * (NBLK // BPC):(bb + 1) * (NBLK // BPC)],
                        axis=AXIS.X, op=ALU.add)

            # combine the 8 gathered partials and broadcast to 128 partitions
            # in one small fp32 matmul: ones8^T[128,8] @ sg8[:,0] -> [128,1]
            psg = sps.tile([128, RB], F32, tag="s")
            nc.tensor.matmul(psg[:, 0:1], lhsT=ones8[:], rhs=sg8[:, 0:1],
                             start=True, stop=True)
            sg = cpool.tile([128, 1], F32, tag="sg")
            nc.scalar.copy(out=sg[:], in_=psg[:, 0:1])

            rcp = cpool.tile([128, 1], F32, tag="rcp")
            nc.vector.tensor_scalar_add(out=rcp[:], in0=sg[:], scalar1=EPS)
            nc.vector.reciprocal(out=rcp[:], in_=rcp[:])
            nc.vector.tensor_scalar_mul(out=out_sb[:], in0=out_sb[:],
                                        scalar1=rcp[:])

            nc.sync.dma_start(out=out.ap(), in_=out_sb[:])

    nc.compile()
    return nc


def _get_compiled(use_bias: bool):
    key = ("nc", use_bias)
    if key not in _CACHE:
        _CACHE[key] = _build(use_bias)
    return _CACHE[key]


def _make_in_maps(x, W, b, u):
    Wc = np.ascontiguousarray(
        np.asarray(W, np.float32).astype(ml_dtypes.bfloat16))
    bc = np.ascontiguousarray(np.asarray(b, np.float32).reshape(NKC, 128).T)
    u_cols = np.asarray(u, np.float32).reshape(NKC, 128).T  # [128, NKC]
    urc = np.ascontiguousarray(
        np.broadcast_to(u_cols[:, :, None], (128, NKC, 128))
    ).astype(ml_dtypes.bfloat16)
    in_maps = []
    for c in range(N_CORES):
        xs = np.asarray(x[BPC * c:BPC * (c + 1)], np.float32).reshape(R, F).T
        xsb = np.ascontiguousarray(
            xs.reshape(NKC, 128, NBLK, RB).transpose(2, 1, 0, 3)
        ).astype(ml_dtypes.bfloat16).reshape(NBLK * 128, CB)
        in_maps.append({"xb": xsb, "w": Wc, "b2": bc, "ur": urc})
    return in_maps


def _unshard_out(o):
    # [128, BPC, NKC] -> [BPC, F]: out[b, kc*128+p] = o[p, b, kc]
    return np.ascontiguousarray(
        o.reshape(128, BPC, NKC).transpose(1, 2, 0).reshape(BPC, F))


def kernel(x, W, b, u):
    use_bias = bool(np.any(np.asarray(b, np.float32)))
    nc = _get_compiled(use_bias)
    in_maps = _make_in_maps(x, W, b, u)
    res = bass_utils.run_bass_kernel_spmd(
        nc, in_maps, core_ids=list(range(N_CORES)))
    _CACHE["last_results"] = res
    return np.concatenate([_unshard_out(res.results[c]["out"])
                           for c in range(N_CORES)], axis=0)


def kernel_traced(x, W, b, u, **trace_kwargs):
    """Same as kernel() but with NTFF tracing; returns (out, BassKernelResults)."""
    use_bias = bool(np.any(np.asarray(b, np.float32)))
    nc = _get_compiled(use_bias)
    in_maps = _make_in_maps(x, W, b, u)
    res = bass_utils.run_bass_kernel_spmd(
        nc, in_maps, core_ids=list(range(N_CORES)), trace=True, **trace_kwargs)
    _CACHE["last_results"] = res
    out = np.concatenate([_unshard_out(res.results[c]["out"])
                          for c in range(N_CORES)], axis=0)
    return out, res
